# revision 24
# baseline (speedup 1.0000x reference)
"""Trainium2 Bass kernel: grouped MoE expert MLP (nn_ExpertGroup).

Strategy: expert parallelism across 8 NeuronCores. Tokens are sorted by
expert; core e runs expert e's two GEMMs:
    h = relu(x_e @ w_up[e].T) ** 2      (bf16, like the CUDA reference)
    y = h @ w_down[e].T
The host does the (free) token scatter/gather, the bf16 casts, and all
layout swizzles so every device-side DMA is a set of 128 fully
contiguous per-partition runs at peak HBM bandwidth.

Device layout (per core, cap = 1024 local tokens):
  One DRAM "blob" [128, 40960] bf16 packed in exact consumption order:
    [wu_j0 | x_c0 | wu_j1..15 | x_c1 | x_c2 | wd]
  so the input stream is 9 big contiguous DMAs whose completion
  semaphores line up with the PE's weight/activation deadlines.
    wu region: (j, d, col) -> w_up[e][j*128+col, d*128+pi]
    x  region: (c, d, t)   -> x[tok_c+t, d*128+pi]   (chunks 256/256/512)
    wd region: (j, i)      -> w_down[e][i, j*128+pi]
  GEMM1 (c outer, j inner): psum[col, t] += wu[:,j,d].T @ x[:,c,d]
  DVE: relu -> bf16, square -> hsq [128, 16, 1024]
  GEMM2 (t outer): psum[t, i] += hsq[:,j,t128].T @ wd[:,j,ic]
  DVE cast -> y [128, (tt, i)] -> host unswizzle.

The first token chunk is 256 wide so the PE needs only 0.75 MB before
its first real group; PE warm-up matmuls bridge engine-init to data
arrival and keep the clock ramp (1.2 -> 2.4 GHz after 3us busy) off the
critical path. The tail drains the last output tile in 256-col pieces.

Built on bacc.Bacc (not raw Bass): Bacc.compile() legalizes semaphore
waits to the TRN2 limit of one wait per instruction.
"""

import numpy as np
import ml_dtypes

import concourse.bass as bass
import concourse.mybir as mybir
import concourse.tile as tile
from concourse import bacc
from concourse.bass_utils import run_bass_kernel_spmd

T, D, H, E = 8192, 1024, 2048, 8
P = 128
N_CORES = 8
CAP = 1024           # tokens per core per round
N_J = H // P         # 16
N_D = D // P         # 8
CHUNKS = ((0, 512), (512, 512))  # GEMM1 token chunks
N_WARM = 46

# blob element offsets (per partition, bf16 elements). The j=0 weights and
# first x chunk are split into d-halves so the PE's first accumulation
# group can start after only 0.625 MB has landed (the head is DMA-supply
# bound: the 16 queues ramp from ~8us at ~0.35 MB/us aggregate).
OFF_WU0A = 0          # wu j=0 d0-3: (d, col) 4*128
OFF_XC0A = 512        # x c=0 d0-3: (d, t) 4*512
OFF_WU0B = 2560       # wu j=0 d4-7
OFF_XC0B = 3072       # x c=0 d4-7
OFF_WUR = 5120        # wu j=1..15: (j-1, d, col)
OFF_XC1 = 20480       # x chunk c=1: (d, t) 8*512
OFF_WD = 24576        # wd: (j, i) 16*1024
TOT = 40960
# input DMA split points, in stream order: (wu_j0 d0-3 + x_c0 d0-1),
# (x_c0 d2-3), (wu_j0 + x_c0 d4-7), then per-j weights for j1-3 (their
# deadlines are tight right after the supply-bound stream start), then
# 6-j batches, x_c1, wd halves. The first matmul needs only 0.375 MB.
DMA_BOUNDS = (0, 1536, 2560, 5120, 6144, 7168, 8192, 14336, 20480, 24576,
              32768, 40960)


def _wu_off(j: int, d: int) -> int:
    if j == 0:
        return (OFF_WU0A + d * P) if d < 4 else (OFF_WU0B + (d - 4) * P)
    return OFF_WUR + (j - 1) * 1024 + d * P


def _x_off(c: int, w: int, d: int) -> int:
    if c == 0:
        return (OFF_XC0A + d * w) if d < 4 else (OFF_XC0B + (d - 4) * w)
    return OFF_XC1 + d * w


def _ensure_axon_ntff_hook():
    """The container's `antenv` stub lacks `axon_hooks`; if BASS_TRACE=1 is
    set, run_bass_kernel_spmd would crash importing it. Recreate the tiny
    registry and register the ctypes NTFF hook so tracing works (and never
    let this best-effort setup break the kernel)."""
    try:
        import antenv.axon_hooks  # noqa: F401
        return
    except ImportError:
        pass
    try:
        import sys
        import types

        import antenv
        from trn_agent_boot.trn_boot import _ntff_profile_via_ctypes

        mod = types.ModuleType("antenv.axon_hooks")
        mod._hook = _ntff_profile_via_ctypes("/opt/axon/libaxon_pjrt.so")
        mod.set_axon_ntff_profile_hook = lambda h: setattr(mod, "_hook", h)
        mod.get_axon_ntff_profile_hook = lambda: mod._hook
        sys.modules["antenv.axon_hooks"] = mod
        antenv.axon_hooks = mod
    except Exception:
        pass


_ensure_axon_ntff_hook()

_PROGRAM_CACHE: dict[int, "bass.Bass"] = {}
LAST_RESULT = None  # BassKernelResults of the most recent run (for harness use)


def _build_program(cap: int) -> "bass.Bass":
    assert cap == CAP
    bf16 = mybir.dt.bfloat16
    f32 = mybir.dt.float32

    nc = bacc.Bacc("TRN2", debug=False, num_devices=N_CORES)
    blob = nc.dram_tensor("blob", [P, TOT], bf16, kind="ExternalInput")
    y = nc.dram_tensor("y", [P, 8 * D], bf16, kind="ExternalOutput")

    with tile.TileContext(nc) as tc:
        with (
            tc.tile_pool(name="big", bufs=1) as big,
            tc.tile_pool(name="outp", bufs=4) as outp,
            tc.tile_pool(name="actp", bufs=4) as actp,
            tc.tile_pool(name="psum", bufs=7, space="PSUM") as psum,
            tc.tile_pool(name="warmp", bufs=1, space="PSUM") as warmp,
        ):
            blob_sb = big.tile([P, TOT], bf16)
            hsq_sb = big.tile([P, N_J, cap], bf16)

            # PE warm-up: dummy matmuls with no DMA dependencies run while
            # the first input DMAs stream in, keeping the PE busy so the
            # clock ramp (full 2.4 GHz after 3us of continuous busy)
            # overlaps the data wait instead of the real stream.
            warm = big.tile([P, P], bf16)
            nc.gpsimd.memset(warm[:], 0.0)
            wps = warmp.tile([P, P], f32, tag="warm")
            for _ in range(N_WARM):
                nc.tensor.matmul(wps, warm[:], warm[:], start=True, stop=True)

            # Input DMAs: slices of one blob in exact consumption order;
            # each is 128 contiguous per-partition runs.
            for a, b in zip(DMA_BOUNDS[:-1], DMA_BOUNDS[1:]):
                nc.sync.dma_start(out=blob_sb[:, a:b], in_=blob[:, a:b])

            # GEMM1 + relu^2: hsq[j-tile, t]
            for c, (tc0, w) in enumerate(CHUNKS):
                for j in range(N_J):
                    ps = psum.tile([P, 512], f32, tag="ps")
                    for d in range(N_D):
                        wo = _wu_off(j, d)
                        xo = _x_off(c, w, d)
                        nc.tensor.matmul(
                            ps[:, 0:w],
                            blob_sb[:, wo:wo + P],
                            blob_sb[:, xo:xo + w],
                            start=(d == 0),
                            stop=(d == N_D - 1),
                        )
                    hr = actp.tile([P, 512], bf16, tag="hr")
                    nc.vector.tensor_relu(out=hr[:, 0:w], in_=ps[:, 0:w])
                    nc.vector.tensor_mul(
                        out=hsq_sb[:, j, tc0:tc0 + w],
                        in0=hr[:, 0:w],
                        in1=hr[:, 0:w],
                    )

            # GEMM2: y[t, i] = sum_j hsq[j, t].T @ wd[j, i]. Output drains
            # as 3 two-tile DMAs (tt pairs), one single-tile DMA (tt=6),
            # then tt=7 in 512/256/256-col pieces to shorten the tail.
            # Fewer DMA semaphores also shorten the teardown verify chain.
            def g2_group(tt, i0, icw, out_ap):
                ps = psum.tile([P, 512], f32, tag="ps")
                for j in range(N_J):
                    nc.tensor.matmul(
                        ps[:, 0:icw],
                        hsq_sb[:, j, tt * P:(tt + 1) * P],
                        blob_sb[:, OFF_WD + j * D + i0:
                                OFF_WD + j * D + i0 + icw],
                        start=(j == 0),
                        stop=(j == N_J - 1),
                    )
                nc.vector.tensor_copy(out=out_ap, in_=ps[:, 0:icw])

            for tp in range(3):
                yt = outp.tile([P, 2 * D], bf16, tag="yt2")
                for k in range(2):
                    tt = 2 * tp + k
                    for i0 in (0, 512):
                        g2_group(tt, i0, 512,
                                 yt[:, k * D + i0:k * D + i0 + 512])
                nc.sync.dma_start(
                    out=y[:, 2 * tp * D:(2 * tp + 2) * D], in_=yt
                )
            yt6 = outp.tile([P, D], bf16, tag="yt1")
            for i0 in (0, 512):
                g2_group(6, i0, 512, yt6[:, i0:i0 + 512])
            nc.sync.dma_start(out=y[:, 6 * D:7 * D], in_=yt6)
            for i0, icw in ((0, 512), (512, 256), (768, 256)):
                yt7 = outp.tile([P, 512], bf16, tag="yts")
                g2_group(7, i0, icw, yt7[:, 0:icw])
                nc.sync.dma_start(
                    out=y[:, 7 * D + i0:7 * D + i0 + icw],
                    in_=yt7[:, 0:icw],
                )

    nc.compile()
    return nc


def _get_program(cap: int) -> "bass.Bass":
    nc = _PROGRAM_CACHE.get(cap)
    if nc is None:
        nc = _build_program(cap)
        _PROGRAM_CACHE[cap] = nc
    return nc


def kernel(x, num_tokens_per_expert, w_up, w_down, _trace=False):
    global LAST_RESULT
    bf = ml_dtypes.bfloat16
    x = np.asarray(x)
    counts = np.asarray(num_tokens_per_expert).astype(np.int64)
    w_up = np.asarray(w_up)
    w_down = np.asarray(w_down)
    n_tok = x.shape[0]
    assert counts.shape == (E,) and int(counts.sum()) == n_tok
    offsets = np.zeros(E, dtype=np.int64)
    offsets[1:] = np.cumsum(counts)[:-1]

    nc = _get_program(CAP)

    # Work list: split each expert's contiguous token segment into slots of
    # <= CAP tokens; process 8 slots per SPMD round. The uniform T/E = 1024
    # split is exactly one round of 8 slots.
    slots = []
    for e in range(E):
        cnt, off = int(counts[e]), int(offsets[e])
        for s in range(0, cnt, CAP):
            slots.append((e, off + s, min(CAP, cnt - s)))

    w_cache = {}

    def expert_weights(e):
        """Pre-swizzled weight blocks: B [128,(j,d,col)], C [128,(j,i)]."""
        if e not in w_cache:
            wu_p = w_up[e].astype(bf)    # (H, D)
            wd_p = w_down[e].astype(bf)  # (D, H)
            B = np.ascontiguousarray(
                wu_p.reshape(N_J, P, N_D, P).transpose(3, 0, 2, 1)
            ).reshape(P, N_J * D)
            C = np.ascontiguousarray(
                wd_p.reshape(D, N_J, P).transpose(2, 1, 0)
            ).reshape(P, N_J * D)
            w_cache[e] = (B, C)
        return w_cache[e]

    out = np.zeros((n_tok, D), dtype=x.dtype)
    zero_map = None
    for r0 in range(0, len(slots), N_CORES):
        round_slots = slots[r0:r0 + N_CORES]
        in_maps = []
        for e, off, cnt in round_slots:
            xs = np.zeros((CAP, D), dtype=bf)
            xs[:cnt] = x[off:off + cnt].astype(bf)
            B, C = expert_weights(e)
            blob = np.empty((P, TOT), dtype=bf)
            blob[:, OFF_WU0A:OFF_WU0A + 512] = B[:, 0:512]
            blob[:, OFF_WU0B:OFF_WU0B + 512] = B[:, 512:1024]
            blob[:, OFF_XC0A:OFF_XC0A + 2048] = (
                xs[0:512, 0:512].reshape(512, 4, P)
                .transpose(2, 1, 0).reshape(P, 2048)
            )
            blob[:, OFF_XC0B:OFF_XC0B + 2048] = (
                xs[0:512, 512:1024].reshape(512, 4, P)
                .transpose(2, 1, 0).reshape(P, 2048)
            )
            blob[:, OFF_WUR:OFF_WUR + 15 * 1024] = B[:, 1024:]
            blob[:, OFF_XC1:OFF_XC1 + 4096] = (
                xs[512:1024].reshape(512, N_D, P)
                .transpose(2, 1, 0).reshape(P, 4096)
            )
            blob[:, OFF_WD:OFF_WD + N_J * D] = C
            in_maps.append({"blob": blob})
        while len(in_maps) < N_CORES:  # idle cores in the last round
            if zero_map is None:
                zero_map = {"blob": np.zeros((P, TOT), dtype=bf)}
            in_maps.append(zero_map)

        res = run_bass_kernel_spmd(
            nc, in_maps, core_ids=list(range(N_CORES)), trace=_trace
        )
        LAST_RESULT = res
        for i, (e, off, cnt) in enumerate(round_slots):
            Y = res.results[i]["y"]  # [128, (tt, i)]
            y_loc = Y.reshape(P, CAP // P, D).transpose(1, 0, 2).reshape(CAP, D)
            out[off:off + cnt] = y_loc[:cnt].astype(x.dtype)
    return out


# revision 26
# speedup vs baseline: 1.0105x; 1.0105x over previous
"""Trainium2 Bass kernel: grouped MoE expert MLP (nn_ExpertGroup).

Strategy: expert parallelism across 8 NeuronCores. Tokens are sorted by
expert; core e runs expert e's two GEMMs:
    h = relu(x_e @ w_up[e].T) ** 2      (bf16, like the CUDA reference)
    y = h @ w_down[e].T
The host does the (free) token scatter/gather, the bf16 casts, and all
layout swizzles so every device-side DMA is a set of 128 fully
contiguous per-partition runs at peak HBM bandwidth.

Device layout (per core, cap = 1024 local tokens):
  One DRAM "blob" [128, 40960] bf16 packed in exact consumption order:
    [wu_j0 | x_c0 | wu_j1..15 | x_c1 | x_c2 | wd]
  so the input stream is 9 big contiguous DMAs whose completion
  semaphores line up with the PE's weight/activation deadlines.
    wu region: (j, d, col) -> w_up[e][j*128+col, d*128+pi]
    x  region: (c, d, t)   -> x[tok_c+t, d*128+pi]   (chunks 256/256/512)
    wd region: (j, i)      -> w_down[e][i, j*128+pi]
  GEMM1 (c outer, j inner): psum[col, t] += wu[:,j,d].T @ x[:,c,d]
  DVE: relu -> bf16, square -> hsq [128, 16, 1024]
  GEMM2 (t outer): psum[t, i] += hsq[:,j,t128].T @ wd[:,j,ic]
  DVE cast -> y [128, (tt, i)] -> host unswizzle.

The first token chunk is 256 wide so the PE needs only 0.75 MB before
its first real group; PE warm-up matmuls bridge engine-init to data
arrival and keep the clock ramp (1.2 -> 2.4 GHz after 3us busy) off the
critical path. The tail drains the last output tile in 256-col pieces.

Built on bacc.Bacc (not raw Bass): Bacc.compile() legalizes semaphore
waits to the TRN2 limit of one wait per instruction.
"""

import numpy as np
import ml_dtypes

import concourse.bass as bass
import concourse.mybir as mybir
import concourse.tile as tile
from concourse import bacc
from concourse.bass_utils import run_bass_kernel_spmd

T, D, H, E = 8192, 1024, 2048, 8
P = 128
N_CORES = 8
CAP = 1024           # tokens per core per round
N_J = H // P         # 16
N_D = D // P         # 8
CHUNKS = ((0, 512), (512, 512))  # GEMM1 token chunks
N_WARM = 50

# blob element offsets (per partition, bf16 elements). The j=0 weights and
# first x chunk are split into d-halves so the PE's first accumulation
# group can start after only 0.625 MB has landed (the head is DMA-supply
# bound: the 16 queues ramp from ~8us at ~0.35 MB/us aggregate).
OFF_WU0A = 0          # wu j=0 d0-3: (d, col) 4*128
OFF_XC0A = 512        # x c=0 d0-3: (d, t) 4*512
OFF_WU0B = 2560       # wu j=0 d4-7
OFF_XC0B = 3072       # x c=0 d4-7
OFF_WUR = 5120        # wu j=1..15: (j-1, d, col)
OFF_XC1 = 20480       # x chunk c=1: (d, t) 8*512
OFF_WD = 24576        # wd: (j, i) 16*1024
TOT = 40960
# input DMA split points, in stream order: d-halves of (wu_j0 + x_c0),
# then per-j weights for j1-3 (their deadlines are tight right after the
# supply-bound stream start), then 6-j batches, x_c1, wd halves. Finer
# splits at the head lose: every extra completion semaphore adds ~0.5us
# of straggler-queue latency exposure.
DMA_BOUNDS = (0, 2560, 5120, 6144, 7168, 8192, 14336, 20480, 24576,
              32768, 40960)


def _wu_off(j: int, d: int) -> int:
    if j == 0:
        return (OFF_WU0A + d * P) if d < 4 else (OFF_WU0B + (d - 4) * P)
    return OFF_WUR + (j - 1) * 1024 + d * P


def _x_off(c: int, w: int, d: int) -> int:
    if c == 0:
        return (OFF_XC0A + d * w) if d < 4 else (OFF_XC0B + (d - 4) * w)
    return OFF_XC1 + d * w


def _ensure_axon_ntff_hook():
    """The container's `antenv` stub lacks `axon_hooks`; if BASS_TRACE=1 is
    set, run_bass_kernel_spmd would crash importing it. Recreate the tiny
    registry and register the ctypes NTFF hook so tracing works (and never
    let this best-effort setup break the kernel)."""
    try:
        import antenv.axon_hooks  # noqa: F401
        return
    except ImportError:
        pass
    try:
        import sys
        import types

        import antenv
        from trn_agent_boot.trn_boot import _ntff_profile_via_ctypes

        mod = types.ModuleType("antenv.axon_hooks")
        mod._hook = _ntff_profile_via_ctypes("/opt/axon/libaxon_pjrt.so")
        mod.set_axon_ntff_profile_hook = lambda h: setattr(mod, "_hook", h)
        mod.get_axon_ntff_profile_hook = lambda: mod._hook
        sys.modules["antenv.axon_hooks"] = mod
        antenv.axon_hooks = mod
    except Exception:
        pass


_ensure_axon_ntff_hook()

_PROGRAM_CACHE: dict[int, "bass.Bass"] = {}
LAST_RESULT = None  # BassKernelResults of the most recent run (for harness use)


def _build_program(cap: int) -> "bass.Bass":
    assert cap == CAP
    bf16 = mybir.dt.bfloat16
    f32 = mybir.dt.float32

    nc = bacc.Bacc("TRN2", debug=False, num_devices=N_CORES)
    blob = nc.dram_tensor("blob", [P, TOT], bf16, kind="ExternalInput")
    y = nc.dram_tensor("y", [P, 8 * D], bf16, kind="ExternalOutput")

    with tile.TileContext(nc) as tc:
        with (
            tc.tile_pool(name="big", bufs=1) as big,
            tc.tile_pool(name="outp", bufs=4) as outp,
            tc.tile_pool(name="actp", bufs=4) as actp,
            tc.tile_pool(name="psum", bufs=7, space="PSUM") as psum,
            tc.tile_pool(name="warmp", bufs=1, space="PSUM") as warmp,
        ):
            blob_sb = big.tile([P, TOT], bf16)
            hsq_sb = big.tile([P, N_J, cap], bf16)

            # PE warm-up: dummy matmuls with no DMA dependencies run while
            # the first input DMAs stream in, keeping the PE busy so the
            # clock ramp (full 2.4 GHz after 3us of continuous busy)
            # overlaps the data wait instead of the real stream.
            warm = big.tile([P, P], bf16)
            nc.gpsimd.memset(warm[:], 0.0)
            wps = warmp.tile([P, P], f32, tag="warm")
            for _ in range(N_WARM):
                nc.tensor.matmul(wps, warm[:], warm[:], start=True, stop=True)

            # Input DMAs: slices of one blob in exact consumption order;
            # each is 128 contiguous per-partition runs.
            for a, b in zip(DMA_BOUNDS[:-1], DMA_BOUNDS[1:]):
                nc.sync.dma_start(out=blob_sb[:, a:b], in_=blob[:, a:b])

            # GEMM1 + relu^2: hsq[j-tile, t]
            for c, (tc0, w) in enumerate(CHUNKS):
                for j in range(N_J):
                    ps = psum.tile([P, 512], f32, tag="ps")
                    for d in range(N_D):
                        wo = _wu_off(j, d)
                        xo = _x_off(c, w, d)
                        nc.tensor.matmul(
                            ps[:, 0:w],
                            blob_sb[:, wo:wo + P],
                            blob_sb[:, xo:xo + w],
                            start=(d == 0),
                            stop=(d == N_D - 1),
                        )
                    hr = actp.tile([P, 512], bf16, tag="hr")
                    nc.vector.tensor_relu(out=hr[:, 0:w], in_=ps[:, 0:w])
                    nc.vector.tensor_mul(
                        out=hsq_sb[:, j, tc0:tc0 + w],
                        in0=hr[:, 0:w],
                        in1=hr[:, 0:w],
                    )

            # GEMM2: y[t, i] = sum_j hsq[j, t].T @ wd[j, i]. Output drains
            # as 3 two-tile DMAs (tt pairs), one single-tile DMA (tt=6),
            # then tt=7 in 512/256/256-col pieces to shorten the tail.
            # Fewer DMA semaphores also shorten the teardown verify chain.
            def g2_group(tt, i0, icw, out_ap):
                ps = psum.tile([P, 512], f32, tag="ps")
                for j in range(N_J):
                    nc.tensor.matmul(
                        ps[:, 0:icw],
                        hsq_sb[:, j, tt * P:(tt + 1) * P],
                        blob_sb[:, OFF_WD + j * D + i0:
                                OFF_WD + j * D + i0 + icw],
                        start=(j == 0),
                        stop=(j == N_J - 1),
                    )
                nc.vector.tensor_copy(out=out_ap, in_=ps[:, 0:icw])

            for tp in range(3):
                yt = outp.tile([P, 2 * D], bf16, tag="yt2")
                for k in range(2):
                    tt = 2 * tp + k
                    for i0 in (0, 512):
                        g2_group(tt, i0, 512,
                                 yt[:, k * D + i0:k * D + i0 + 512])
                nc.sync.dma_start(
                    out=y[:, 2 * tp * D:(2 * tp + 2) * D], in_=yt
                )
            yt6 = outp.tile([P, D], bf16, tag="yt1")
            for i0 in (0, 512):
                g2_group(6, i0, 512, yt6[:, i0:i0 + 512])
            nc.sync.dma_start(out=y[:, 6 * D:7 * D], in_=yt6)
            for i0, icw in ((0, 512), (512, 256), (768, 256)):
                yt7 = outp.tile([P, 512], bf16, tag="yts")
                g2_group(7, i0, icw, yt7[:, 0:icw])
                nc.sync.dma_start(
                    out=y[:, 7 * D + i0:7 * D + i0 + icw],
                    in_=yt7[:, 0:icw],
                )

    nc.compile()
    return nc


def _get_program(cap: int) -> "bass.Bass":
    nc = _PROGRAM_CACHE.get(cap)
    if nc is None:
        nc = _build_program(cap)
        _PROGRAM_CACHE[cap] = nc
    return nc


def kernel(x, num_tokens_per_expert, w_up, w_down, _trace=False):
    global LAST_RESULT
    bf = ml_dtypes.bfloat16
    x = np.asarray(x)
    counts = np.asarray(num_tokens_per_expert).astype(np.int64)
    w_up = np.asarray(w_up)
    w_down = np.asarray(w_down)
    n_tok = x.shape[0]
    assert counts.shape == (E,) and int(counts.sum()) == n_tok
    offsets = np.zeros(E, dtype=np.int64)
    offsets[1:] = np.cumsum(counts)[:-1]

    nc = _get_program(CAP)

    # Work list: split each expert's contiguous token segment into slots of
    # <= CAP tokens; process 8 slots per SPMD round. The uniform T/E = 1024
    # split is exactly one round of 8 slots.
    slots = []
    for e in range(E):
        cnt, off = int(counts[e]), int(offsets[e])
        for s in range(0, cnt, CAP):
            slots.append((e, off + s, min(CAP, cnt - s)))

    w_cache = {}

    def expert_weights(e):
        """Pre-swizzled weight blocks: B [128,(j,d,col)], C [128,(j,i)]."""
        if e not in w_cache:
            wu_p = w_up[e].astype(bf)    # (H, D)
            wd_p = w_down[e].astype(bf)  # (D, H)
            B = np.ascontiguousarray(
                wu_p.reshape(N_J, P, N_D, P).transpose(3, 0, 2, 1)
            ).reshape(P, N_J * D)
            C = np.ascontiguousarray(
                wd_p.reshape(D, N_J, P).transpose(2, 1, 0)
            ).reshape(P, N_J * D)
            w_cache[e] = (B, C)
        return w_cache[e]

    out = np.zeros((n_tok, D), dtype=x.dtype)
    zero_map = None
    for r0 in range(0, len(slots), N_CORES):
        round_slots = slots[r0:r0 + N_CORES]
        in_maps = []
        for e, off, cnt in round_slots:
            xs = np.zeros((CAP, D), dtype=bf)
            xs[:cnt] = x[off:off + cnt].astype(bf)
            B, C = expert_weights(e)
            blob = np.empty((P, TOT), dtype=bf)
            blob[:, OFF_WU0A:OFF_WU0A + 512] = B[:, 0:512]
            blob[:, OFF_WU0B:OFF_WU0B + 512] = B[:, 512:1024]
            blob[:, OFF_XC0A:OFF_XC0A + 2048] = (
                xs[0:512, 0:512].reshape(512, 4, P)
                .transpose(2, 1, 0).reshape(P, 2048)
            )
            blob[:, OFF_XC0B:OFF_XC0B + 2048] = (
                xs[0:512, 512:1024].reshape(512, 4, P)
                .transpose(2, 1, 0).reshape(P, 2048)
            )
            blob[:, OFF_WUR:OFF_WUR + 15 * 1024] = B[:, 1024:]
            blob[:, OFF_XC1:OFF_XC1 + 4096] = (
                xs[512:1024].reshape(512, N_D, P)
                .transpose(2, 1, 0).reshape(P, 4096)
            )
            blob[:, OFF_WD:OFF_WD + N_J * D] = C
            in_maps.append({"blob": blob})
        while len(in_maps) < N_CORES:  # idle cores in the last round
            if zero_map is None:
                zero_map = {"blob": np.zeros((P, TOT), dtype=bf)}
            in_maps.append(zero_map)

        res = run_bass_kernel_spmd(
            nc, in_maps, core_ids=list(range(N_CORES)), trace=_trace
        )
        LAST_RESULT = res
        for i, (e, off, cnt) in enumerate(round_slots):
            Y = res.results[i]["y"]  # [128, (tt, i)]
            y_loc = Y.reshape(P, CAP // P, D).transpose(1, 0, 2).reshape(CAP, D)
            out[off:off + cnt] = y_loc[:cnt].astype(x.dtype)
    return out


# revision 28
# speedup vs baseline: 1.0140x; 1.0036x over previous
"""Trainium2 Bass kernel: grouped MoE expert MLP (nn_ExpertGroup).

Strategy: expert parallelism across 8 NeuronCores. Tokens are sorted by
expert; core e runs expert e's two GEMMs:
    h = relu(x_e @ w_up[e].T) ** 2      (bf16, like the CUDA reference)
    y = h @ w_down[e].T
The host does the (free) token scatter/gather, the bf16 casts, and all
layout swizzles so every device-side DMA is a set of 128 fully
contiguous per-partition runs at peak HBM bandwidth.

Device layout (per core, cap = 1024 local tokens):
  One DRAM "blob" [128, 40960] bf16 packed in exact consumption order:
    [wu_j0 | x_c0 | wu_j1..15 | x_c1 | x_c2 | wd]
  so the input stream is 9 big contiguous DMAs whose completion
  semaphores line up with the PE's weight/activation deadlines.
    wu region: (j, d, col) -> w_up[e][j*128+col, d*128+pi]
    x  region: (c, d, t)   -> x[tok_c+t, d*128+pi]   (chunks 256/256/512)
    wd region: (j, i)      -> w_down[e][i, j*128+pi]
  GEMM1 (c outer, j inner): psum[col, t] += wu[:,j,d].T @ x[:,c,d]
  DVE: relu -> bf16, square -> hsq [128, 16, 1024]
  GEMM2 (t outer): psum[t, i] += hsq[:,j,t128].T @ wd[:,j,ic]
  DVE cast -> y [128, (tt, i)] -> host unswizzle.

The first token chunk is 256 wide so the PE needs only 0.75 MB before
its first real group; PE warm-up matmuls bridge engine-init to data
arrival and keep the clock ramp (1.2 -> 2.4 GHz after 3us busy) off the
critical path. The tail drains the last output tile in 256-col pieces.

Built on bacc.Bacc (not raw Bass): Bacc.compile() legalizes semaphore
waits to the TRN2 limit of one wait per instruction.
"""

import numpy as np
import ml_dtypes

import concourse.bass as bass
import concourse.mybir as mybir
import concourse.tile as tile
from concourse import bacc
from concourse.bass_utils import run_bass_kernel_spmd

T, D, H, E = 8192, 1024, 2048, 8
P = 128
N_CORES = 8
CAP = 1024           # tokens per core per round
N_J = H // P         # 16
N_D = D // P         # 8
CHUNKS = ((0, 512), (512, 512))  # GEMM1 token chunks
N_WARM = 50

# blob element offsets (per partition, bf16 elements). The j=0 weights and
# first x chunk are split into d-halves so the PE's first accumulation
# group can start after only 0.625 MB has landed (the head is DMA-supply
# bound: the 16 queues ramp from ~8us at ~0.35 MB/us aggregate).
OFF_WU0A = 0          # wu j=0 d0-3: (d, col) 4*128
OFF_XC0A = 512        # x c=0 d0-3: (d, t) 4*512
OFF_WU0B = 2560       # wu j=0 d4-7
OFF_XC0B = 3072       # x c=0 d4-7
OFF_WUR = 5120        # wu j=1..15: (j-1, d, col)
OFF_XC1 = 20480       # x chunk c=1: (d, t) 8*512
OFF_WD = 24576        # wd: (j, i) 16*1024
TOT = 40960
# input DMA split points, in stream order: d-halves of (wu_j0 + x_c0),
# then per-j weights for j1-3 (their deadlines are tight right after the
# supply-bound stream start), then 6-j batches, x_c1, wd halves. Finer
# splits at the head lose: every extra completion semaphore adds ~0.5us
# of straggler-queue latency exposure.
DMA_BOUNDS = (0, 2560, 5120, 6144, 7168, 8192, 14336, 20480, 24576,
              32768, 40960)


def _wu_off(j: int, d: int) -> int:
    if j == 0:
        return (OFF_WU0A + d * P) if d < 4 else (OFF_WU0B + (d - 4) * P)
    return OFF_WUR + (j - 1) * 1024 + d * P


def _x_off(c: int, w: int, d: int) -> int:
    if c == 0:
        return (OFF_XC0A + d * w) if d < 4 else (OFF_XC0B + (d - 4) * w)
    return OFF_XC1 + d * w


def _ensure_axon_ntff_hook():
    """The container's `antenv` stub lacks `axon_hooks`; if BASS_TRACE=1 is
    set, run_bass_kernel_spmd would crash importing it. Recreate the tiny
    registry and register the ctypes NTFF hook so tracing works (and never
    let this best-effort setup break the kernel)."""
    try:
        import antenv.axon_hooks  # noqa: F401
        return
    except ImportError:
        pass
    try:
        import sys
        import types

        import antenv
        from trn_agent_boot.trn_boot import _ntff_profile_via_ctypes

        mod = types.ModuleType("antenv.axon_hooks")
        mod._hook = _ntff_profile_via_ctypes("/opt/axon/libaxon_pjrt.so")
        mod.set_axon_ntff_profile_hook = lambda h: setattr(mod, "_hook", h)
        mod.get_axon_ntff_profile_hook = lambda: mod._hook
        sys.modules["antenv.axon_hooks"] = mod
        antenv.axon_hooks = mod
    except Exception:
        pass


_ensure_axon_ntff_hook()

_PROGRAM_CACHE: dict[int, "bass.Bass"] = {}
LAST_RESULT = None  # BassKernelResults of the most recent run (for harness use)


def _build_program(cap: int) -> "bass.Bass":
    assert cap == CAP
    bf16 = mybir.dt.bfloat16
    f32 = mybir.dt.float32

    nc = bacc.Bacc("TRN2", debug=False, num_devices=N_CORES)
    blob = nc.dram_tensor("blob", [P, TOT], bf16, kind="ExternalInput")
    y = nc.dram_tensor("y", [P, 8 * D], bf16, kind="ExternalOutput")

    with tile.TileContext(nc) as tc:
        with (
            tc.tile_pool(name="big", bufs=1) as big,
            tc.tile_pool(name="outp", bufs=2) as outp,
            tc.tile_pool(name="actp", bufs=3) as actp,
            tc.tile_pool(name="psum", bufs=5, space="PSUM") as psum,
            tc.tile_pool(name="warmp", bufs=1, space="PSUM") as warmp,
        ):
            blob_sb = big.tile([P, TOT], bf16)
            hsq_sb = big.tile([P, N_J, cap], bf16)

            # PE warm-up: dummy matmuls with no DMA dependencies run while
            # the first input DMAs stream in, keeping the PE busy so the
            # clock ramp (full 2.4 GHz after 3us of continuous busy)
            # overlaps the data wait instead of the real stream.
            warm = big.tile([P, P], bf16)
            nc.vector.memset(warm[:], 0.0)
            wps = warmp.tile([P, P], f32, tag="warm")
            for _ in range(N_WARM):
                nc.tensor.matmul(wps, warm[:], warm[:], start=True, stop=True)

            # Input DMAs: slices of one blob in exact consumption order;
            # each is 128 contiguous per-partition runs.
            for a, b in zip(DMA_BOUNDS[:-1], DMA_BOUNDS[1:]):
                nc.sync.dma_start(out=blob_sb[:, a:b], in_=blob[:, a:b])

            # GEMM1 + relu^2: hsq[j-tile, t]
            for c, (tc0, w) in enumerate(CHUNKS):
                for j in range(N_J):
                    ps = psum.tile([P, 512], f32, tag="ps")
                    for d in range(N_D):
                        wo = _wu_off(j, d)
                        xo = _x_off(c, w, d)
                        nc.tensor.matmul(
                            ps[:, 0:w],
                            blob_sb[:, wo:wo + P],
                            blob_sb[:, xo:xo + w],
                            start=(d == 0),
                            stop=(d == N_D - 1),
                        )
                    hr = actp.tile([P, 512], bf16, tag="hr")
                    nc.vector.tensor_relu(out=hr[:, 0:w], in_=ps[:, 0:w])
                    nc.vector.tensor_mul(
                        out=hsq_sb[:, j, tc0:tc0 + w],
                        in0=hr[:, 0:w],
                        in1=hr[:, 0:w],
                    )

            # GEMM2: y[t, i] = sum_j hsq[j, t].T @ wd[j, i]. Output drains
            # as 3 two-tile DMAs (tt pairs), one single-tile DMA (tt=6),
            # then tt=7 in 512/256/256-col pieces to shorten the tail.
            # Fewer DMA semaphores also shorten the teardown verify chain.
            def g2_group(tt, i0, icw, out_ap):
                ps = psum.tile([P, 512], f32, tag="ps")
                for j in range(N_J):
                    nc.tensor.matmul(
                        ps[:, 0:icw],
                        hsq_sb[:, j, tt * P:(tt + 1) * P],
                        blob_sb[:, OFF_WD + j * D + i0:
                                OFF_WD + j * D + i0 + icw],
                        start=(j == 0),
                        stop=(j == N_J - 1),
                    )
                nc.vector.tensor_copy(out=out_ap, in_=ps[:, 0:icw])

            for tp in range(3):
                yt = outp.tile([P, 2 * D], bf16, tag="yt2")
                for k in range(2):
                    tt = 2 * tp + k
                    for i0 in (0, 512):
                        g2_group(tt, i0, 512,
                                 yt[:, k * D + i0:k * D + i0 + 512])
                nc.sync.dma_start(
                    out=y[:, 2 * tp * D:(2 * tp + 2) * D], in_=yt
                )
            yt6 = outp.tile([P, D], bf16, tag="yt1")
            for i0 in (0, 512):
                g2_group(6, i0, 512, yt6[:, i0:i0 + 512])
            nc.sync.dma_start(out=y[:, 6 * D:7 * D], in_=yt6)
            for i0, icw in ((0, 512), (512, 256), (768, 256)):
                yt7 = outp.tile([P, 512], bf16, tag="yts")
                g2_group(7, i0, icw, yt7[:, 0:icw])
                nc.sync.dma_start(
                    out=y[:, 7 * D + i0:7 * D + i0 + icw],
                    in_=yt7[:, 0:icw],
                )

    nc.compile()
    return nc


def _get_program(cap: int) -> "bass.Bass":
    nc = _PROGRAM_CACHE.get(cap)
    if nc is None:
        nc = _build_program(cap)
        _PROGRAM_CACHE[cap] = nc
    return nc


def kernel(x, num_tokens_per_expert, w_up, w_down, _trace=False):
    global LAST_RESULT
    bf = ml_dtypes.bfloat16
    x = np.asarray(x)
    counts = np.asarray(num_tokens_per_expert).astype(np.int64)
    w_up = np.asarray(w_up)
    w_down = np.asarray(w_down)
    n_tok = x.shape[0]
    assert counts.shape == (E,) and int(counts.sum()) == n_tok
    offsets = np.zeros(E, dtype=np.int64)
    offsets[1:] = np.cumsum(counts)[:-1]

    nc = _get_program(CAP)

    # Work list: split each expert's contiguous token segment into slots of
    # <= CAP tokens; process 8 slots per SPMD round. The uniform T/E = 1024
    # split is exactly one round of 8 slots.
    slots = []
    for e in range(E):
        cnt, off = int(counts[e]), int(offsets[e])
        for s in range(0, cnt, CAP):
            slots.append((e, off + s, min(CAP, cnt - s)))

    w_cache = {}

    def expert_weights(e):
        """Pre-swizzled weight blocks: B [128,(j,d,col)], C [128,(j,i)]."""
        if e not in w_cache:
            wu_p = w_up[e].astype(bf)    # (H, D)
            wd_p = w_down[e].astype(bf)  # (D, H)
            B = np.ascontiguousarray(
                wu_p.reshape(N_J, P, N_D, P).transpose(3, 0, 2, 1)
            ).reshape(P, N_J * D)
            C = np.ascontiguousarray(
                wd_p.reshape(D, N_J, P).transpose(2, 1, 0)
            ).reshape(P, N_J * D)
            w_cache[e] = (B, C)
        return w_cache[e]

    out = np.zeros((n_tok, D), dtype=x.dtype)
    zero_map = None
    for r0 in range(0, len(slots), N_CORES):
        round_slots = slots[r0:r0 + N_CORES]
        in_maps = []
        for e, off, cnt in round_slots:
            xs = np.zeros((CAP, D), dtype=bf)
            xs[:cnt] = x[off:off + cnt].astype(bf)
            B, C = expert_weights(e)
            blob = np.empty((P, TOT), dtype=bf)
            blob[:, OFF_WU0A:OFF_WU0A + 512] = B[:, 0:512]
            blob[:, OFF_WU0B:OFF_WU0B + 512] = B[:, 512:1024]
            blob[:, OFF_XC0A:OFF_XC0A + 2048] = (
                xs[0:512, 0:512].reshape(512, 4, P)
                .transpose(2, 1, 0).reshape(P, 2048)
            )
            blob[:, OFF_XC0B:OFF_XC0B + 2048] = (
                xs[0:512, 512:1024].reshape(512, 4, P)
                .transpose(2, 1, 0).reshape(P, 2048)
            )
            blob[:, OFF_WUR:OFF_WUR + 15 * 1024] = B[:, 1024:]
            blob[:, OFF_XC1:OFF_XC1 + 4096] = (
                xs[512:1024].reshape(512, N_D, P)
                .transpose(2, 1, 0).reshape(P, 4096)
            )
            blob[:, OFF_WD:OFF_WD + N_J * D] = C
            in_maps.append({"blob": blob})
        while len(in_maps) < N_CORES:  # idle cores in the last round
            if zero_map is None:
                zero_map = {"blob": np.zeros((P, TOT), dtype=bf)}
            in_maps.append(zero_map)

        res = run_bass_kernel_spmd(
            nc, in_maps, core_ids=list(range(N_CORES)), trace=_trace
        )
        LAST_RESULT = res
        for i, (e, off, cnt) in enumerate(round_slots):
            Y = res.results[i]["y"]  # [128, (tt, i)]
            y_loc = Y.reshape(P, CAP // P, D).transpose(1, 0, 2).reshape(CAP, D)
            out[off:off + cnt] = y_loc[:cnt].astype(x.dtype)
    return out
